# revision 24
# baseline (speedup 1.0000x reference)
"""EquivariantAttention Trainium2 kernel (v3 — pipelined).

B=2, L=2048, D=512, H=8, HD=64 over 8 NeuronCores.
Head-parallel attention (core c owns head c, both batches), AllToAll to
sequence-shard the output projection (core c owns q-window [256c, 256c+256)).

Math notes:
  Qi . Ki = ||Q_l||*||K_m|| + Q_l^T C K_m,  C = basis_q[:63].T @ basis_k[:63]
  -> scores^T computed transposed ([k, q]) with 66-row operands:
     qt = [Qt ; ||Q|| ; ||Q||]   kt = [C Kt ; ||K|| ; -muk]
  so the contraction yields qck + ||Q||*(||K|| - muk) with no separate
  centering op.  The softmax denominator comes from an appended ones-row in V
  (row 64 of U~); softmax is max-free (scores bounded for this problem).

Structure:
  - x is DMA'd per 512-column slice ([128, 4, 512] rearranged loads) and the
    projection + attention for each slice run back-to-back, so the PE starts
    ~4us in and tracks the x DMA.
  - squares on all 128 partitions straight from the QK PSUM tile (bias-free
    fast path) + one paired ones-reduce matmul; one Sqrt writes both nq rows.
  - ACT-order dependency edges force [sqrts(b0)][exps(b0)][sqrts(b1)][exps(b1)]
    so the activation-table (Sqrt set <-> Exp set) loads exactly 4 times.
  - exp outputs fp16; V/PV and the output projection run fp16 (1 cycle/row).
  - gpsimd queue carries ONLY const DMAs + the collectives: anything else
    there would queue behind the first collective's input wait.
  - per-batch AllToAll issued as soon as that batch's attention is done,
    overlapping the other batch's compute.
"""

import sys

sys.path.insert(0, "/opt/trn_rl_repo")

import numpy as np

import concourse.bass as bass  # noqa: F401  (AP helpers)
import concourse.tile as tile
from concourse import bacc, mybir
from concourse.bass_utils import run_bass_kernel_spmd
from concourse.tile import add_dep_helper

F32 = mybir.dt.float32
F32R = mybir.dt.float32r
F16 = mybir.dt.float16
EXP = mybir.ActivationFunctionType.Exp
SQRT = mybir.ActivationFunctionType.Sqrt

B, L, D, H, HD = 2, 2048, 512, 8, 64
NC = 8
LW = L // NC          # 256: per-core q-window for the output projection
NL = 4                # l-slices of 512 per batch
NK = L // 128         # 16 k-tiles per batch
SHIFT = -20.0         # softmax global shift; exp stays f32r (score range
                      # ~[-221, 332] spans ~69 nats -- far beyond fp16)


def _build(causal: bool, repeat: int = 1):
    nc = bacc.Bacc("TRN2", target_bir_lowering=False, debug=False,
                   enable_asserts=True, num_devices=NC)

    xt = nc.dram_tensor("xt", [B, D, L], F32R, kind="ExternalInput")
    wqk4 = nc.dram_tensor("wqk4", [128, 512], F32R, kind="ExternalInput")
    wv4 = nc.dram_tensor("wv4", [128, 256], F32R, kind="ExternalInput")
    wo4h = nc.dram_tensor("wo4h", [128, 2048], F16, kind="ExternalInput")
    cmt = nc.dram_tensor("cmt", [HD, HD], F32R, kind="ExternalInput")
    on3 = nc.dram_tensor("on3", [128, 3], F32R, kind="ExternalInput")
    onr = nc.dram_tensor("onr", [1, HD], F32R, kind="ExternalInput")
    onp = nc.dram_tensor("onp", [128, 1], F32R, kind="ExternalInput")
    mukr = nc.dram_tensor("mukr", [1, L], F32R, kind="ExternalInput")
    idm = nc.dram_tensor("idm", [128, 128], F32R, kind="ExternalInput")
    if not causal:
        maskf = nc.dram_tensor("maskf", [L, L], F32, kind="ExternalInput")
    bq = nc.dram_tensor("bq", [HD, 1], F32, kind="ExternalInput")
    bk = nc.dram_tensor("bk", [HD, 1], F32, kind="ExternalInput")
    bv = nc.dram_tensor("bv", [128, HD], F32, kind="ExternalInput")
    bo4 = nc.dram_tensor("bo4", [128, 4], F32, kind="ExternalInput")
    yts = nc.dram_tensor("yts", [B, D, LW], F32, kind="ExternalOutput")

    from contextlib import ExitStack
    with tile.TileContext(nc) as tc, ExitStack() as ctx:
            ec = ctx.enter_context
            const = ec(tc.tile_pool(name="const", bufs=1))
            xtp = ec(tc.tile_pool(name="xtp", bufs=6))
            qtp = ec(tc.tile_pool(name="qtp", bufs=2))
            ktp = ec(tc.tile_pool(name="ktp", bufs=2))
            krawp = ec(tc.tile_pool(name="krawp", bufs=2))
            sqp = ec(tc.tile_pool(name="sqp", bufs=2))
            vttp = ec(tc.tile_pool(name="vttp", bufs=2))
            vp = ec(tc.tile_pool(name="vp", bufs=2))
            expp = ec(tc.tile_pool(name="expp", bufs=20))
            uscp = ec(tc.tile_pool(name="uscp", bufs=3))
            rzp = ec(tc.tile_pool(name="rzp", bufs=2))
            rvp = ec(tc.tile_pool(name="rvp", bufs=4))
            ytp = ec(tc.tile_pool(name="ytp", bufs=3))
            mskp = ec(tc.tile_pool(name="mskp", bufs=3 if not causal else 1))
            pb = ec(tc.tile_pool(name="pb", bufs=3, space="PSUM"))
            sp = ec(tc.tile_pool(name="sp", bufs=3, space="PSUM"))
            up = ec(tc.tile_pool(name="up", bufs=2, space="PSUM"))
            dram = ec(tc.tile_pool(name="dram", bufs=1, space="DRAM"))

            # ---- constants into SBUF.  scalar HWDGE queue carries the
            # weights needed first; gpsimd SWDGE the rest; sync queue is
            # reserved for x / sends / rv / yts so x starts immediately. ----
            wqk_sb = const.tile([128, 4, 128], F32R)
            wv_sb = const.tile([128, 4, HD], F32R)
            wo_sb = const.tile([128, 4, D], F16)
            cm_sb = const.tile([HD, HD], F32R)
            on3_sb = const.tile([128, 3], F32R)
            onr_sb = const.tile([1, HD], F32R)
            onp_sb = const.tile([128, 1], F32R)
            ident_sb = const.tile([128, 128], F32R)
            shift_sb = const.tile([128, 1], F32)

            nc.vector.memset(shift_sb[:, :], SHIFT)
            bq_sb = const.tile([HD, 1], F32)
            bk_sb = const.tile([HD, 1], F32)
            bv_sb = const.tile([128, HD], F32)
            bo_sb = const.tile([128, 4], F32)
            # consts on the scalar HWDGE queue: their SEQ dispatches finish
            # before ACT's first compute op; the sync queue starts x at ~2us
            nc.scalar.dma_start(out=wqk_sb[:, :, :],
                                in_=wqk4[:, :].rearrange("p (c m) -> p c m", c=4))
            nc.scalar.dma_start(out=bq_sb[:, :], in_=bq[:, :])
            nc.scalar.dma_start(out=bk_sb[:, :], in_=bk[:, :])
            nc.scalar.dma_start(out=on3_sb[:, :], in_=on3[:, :])
            nc.scalar.dma_start(out=cm_sb[:, :], in_=cmt[:, :])
            # bulk / later-needed consts on the scalar HWDGE queue
            nc.scalar.dma_start(out=wv_sb[:, :, :],
                                in_=wv4[:, :].rearrange("p (c m) -> p c m", c=4))
            nc.scalar.dma_start(out=ident_sb[:, :], in_=idm[:, :])
            nc.scalar.dma_start(out=bv_sb[:, :], in_=bv[:, :])
            nc.scalar.dma_start(out=onr_sb[:, :], in_=onr[:, :])
            nc.scalar.dma_start(out=onp_sb[:, :], in_=onp[:, :])
            nc.scalar.dma_start(out=wo_sb[:, :, :],
                                in_=wo4h[:, :].rearrange("p (c m) -> p c m", c=4))
            nc.scalar.dma_start(out=bo_sb[:, :], in_=bo4[:, :])

            for _rep in range(repeat):
                send = [dram.tile([NC, HD, LW], F16, tag=f"send{b}", name=f"send{b}") for b in range(B)]
                recv = [dram.tile([NC, HD, LW], F16, tag=f"recv{b}", name=f"recv{b}") for b in range(B)]

                # per-batch tiles + their mukr rows first (tiny, gates the
                # score matmuls), then all x loads on the sync queue
                state = {}
                for b in range(B):
                    qt = qtp.tile([HD + 2, L], F32R)   # [Qt ; ||Q|| ; ||Q||]
                    kt = ktp.tile([HD + 2, L], F32R)   # [C Kt ; ||K|| ; -muk]
                    kraw = krawp.tile([HD, L], F32R)
                    vtt = vttp.tile([HD, L], F32R)
                    vt = vp.tile([128, NK, HD + 1], F32R)
                    nc.sync.dma_start(out=kt[HD + 1:HD + 2, :], in_=mukr[:, :])
                    state[b] = (qt, kt, kraw, vtt, vt)
                xts = {}
                for b in range(B):
                    for ls in range(NL):
                        t = xtp.tile([128, 4, 512], F32R, tag="xt")
                        nc.sync.dma_start(
                            out=t[:, :, :],
                            in_=xt[b, :, 512 * ls:512 * (ls + 1)]
                                .rearrange("(c p) m -> p c m", p=128))
                        xts[(b, ls)] = t

                sqrt_insts = {0: [], 1: []}
                exp_insts = {0: [], 1: []}

                def pp_qk(b, ls):
                    qt, kt, kraw, vtt, vt = state[b]
                    s = slice(512 * ls, 512 * (ls + 1))
                    xls = xts[(b, ls)]
                    qk_ps = pb.tile([128, 512], F32, tag="pb")
                    for dc in range(4):
                        nc.tensor.matmul(qk_ps[:, :], wqk_sb[:, dc, :],
                                         xls[:, dc, :],
                                         start=(dc == 0), stop=(dc == 3))
                    nc.vector.tensor_scalar_add(qt[0:HD, s], qk_ps[0:HD, :],
                                                bq_sb[:, 0:1])
                    nc.vector.tensor_scalar_add(kraw[:, s], qk_ps[HD:128, :],
                                                bk_sb[:, 0:1])
                    sq2 = sqp.tile([128, 512], F32R, tag="sq")
                    nc.vector.tensor_mul(sq2[0:HD, :], qt[0:HD, s], qt[0:HD, s])
                    nc.vector.tensor_mul(sq2[HD:128, :], kraw[:, s], kraw[:, s])
                    # paired column-sums; one Sqrt fills both nq rows of qt,
                    # one fills kt's nk row (PSUM reads must start at part 0).
                    ssqq_ps = sp.tile([2, 512], F32, tag="sp")
                    nc.tensor.matmul(ssqq_ps[:, :], on3_sb[:, 0:2], sq2[:, :],
                                     start=True, stop=True)
                    ssqk_ps = sp.tile([1, 512], F32, tag="sp")
                    nc.tensor.matmul(ssqk_ps[:, :], on3_sb[:, 2:3], sq2[:, :],
                                     start=True, stop=True)
                    i1 = nc.scalar.activation(qt[HD:HD + 2, s], ssqq_ps[:, :], SQRT)
                    i2 = nc.scalar.activation(kt[HD:HD + 1, s], ssqk_ps[:, :], SQRT)
                    sqrt_insts[b] += [i1, i2]
                    # CK projection (kt needed by the first score tiles)
                    ck_ps = pb.tile([128, 512], F32, tag="pb")
                    nc.tensor.matmul(ck_ps[0:HD, :], cm_sb[:, :], kraw[:, s],
                                     start=True, stop=True)
                    # b0: ACT is idle during its projection phase; b1's kt
                    # copies run while ACT streams b0's exps -> DVE instead
                    if b == 0:
                        nc.scalar.copy(kt[0:HD, s], ck_ps[0:HD, :])
                    else:
                        nc.vector.tensor_copy(kt[0:HD, s], ck_ps[0:HD, :])

                def pp_v(b, ls):
                    qt, kt, kraw, vtt, vt = state[b]
                    s = slice(512 * ls, 512 * (ls + 1))
                    xls = xts[(b, ls)]
                    # V (transposed projection), then PE transpose to rows
                    vt_ps = pb.tile([128, 512], F32, tag="pb")
                    for dc in range(4):
                        nc.tensor.matmul(vt_ps[0:HD, :], wv_sb[:, dc, :],
                                         xls[:, dc, :],
                                         start=(dc == 0), stop=(dc == 3))
                    nc.vector.tensor_copy(vtt[:, s], vt_ps[0:HD, :])
                    tp_ps = pb.tile([128, 4, HD], F32, tag="pb")
                    for j in range(4):
                        lt = 4 * ls + j
                        nc.tensor.transpose(tp_ps[:, j, :].bitcast(F32R),
                                            vtt[:, 128 * lt:128 * (lt + 1)],
                                            ident_sb[0:HD, 0:HD])
                    for j in range(4):
                        nc.vector.tensor_add(vt[:, 4 * ls + j, 0:HD],
                                             tp_ps[:, j, :], bv_sb[:, :])
                        nc.vector.tensor_copy(vt[:, 4 * ls + j, HD:HD + 1],
                                              onp_sb[:, :])

                ex_tiles = {}

                def _kis(n):
                    kmax = 4 * (n + 1) if causal else NK
                    return list(range(kmax))

                def _win(n, ki):
                    # diagonal k-tiles only need q-cols >= 128r within the
                    # window; ki==0/first is always full width (start=True)
                    # so trimmed tiles overwrite cleanly.
                    lo = 0
                    if causal and 4 * n <= ki <= 4 * n + 3:
                        lo = 128 * (ki - 4 * n)
                    return slice(lo, 512)

                def att_scores(b, n):
                    qt, kt, kraw, vtt, vt = state[b]
                    qs = slice(512 * n, 512 * (n + 1))
                    for ki in _kis(n):
                        w = _win(n, ki)
                        st_ps = sp.tile([128, 512], F32, tag="sp")
                        nc.tensor.matmul(st_ps[:, w],
                                         kt[:, 128 * ki:128 * (ki + 1)],
                                         qt[:, qs][:, w], start=True, stop=True)
                        if not causal:
                            mld = mskp.tile([128, 512], F32)
                            nc.sync.dma_start(out=mld[:, :],
                                              in_=maskf[128 * ki:128 * (ki + 1), qs])
                            nc.vector.tensor_add(st_ps[:, :], st_ps[:, :], mld[:, :])
                        ex = expp.tile([128, 512], F32R)
                        ie = nc.scalar.activation(ex[:, w], st_ps[:, w], EXP,
                                                  scale=0.125,
                                                  bias=shift_sb[:, 0:1])
                        exp_insts[b].append(ie)
                        if causal and 4 * n <= ki <= 4 * n + 3:
                            # zero the strictly-upper triangle of the diagonal
                            # block on the idle gpsimd engine (post-exp)
                            r = ki - 4 * n
                            ds_ = slice(128 * r, 128 * (r + 1))
                            nc.gpsimd.affine_select(
                                ex[:, ds_], ex[:, ds_],
                                pattern=[[1, 128]],
                                compare_op=mybir.AluOpType.is_ge,
                                fill=0.0, base=0, channel_multiplier=-1)
                        ex_tiles[(b, n, ki)] = ex

                def att_pv(b, n):
                    qt, kt, kraw, vtt, vt = state[b]
                    kis = _kis(n)
                    first_ki, last_ki = kis[0], kis[-1]
                    u_ps = up.tile([HD + 1, 512], F32, tag="up")
                    for ki in kis:
                        w = _win(n, ki)
                        ex = ex_tiles.pop((b, n, ki))
                        nc.tensor.matmul(u_ps[:, w], vt[:, ki, :], ex[:, w],
                                         start=(ki == first_ki),
                                         stop=(ki == last_ki))
                    rz = rzp.tile([1, 512], F32R, tag="rz")
                    with nc.allow_low_precision(reason="f32r rounding of softmax denom"):
                        nc.vector.reciprocal(rz[:, :], u_ps[HD:HD + 1, :])
                    zb_ps = pb.tile([HD, 512], F32, tag="pb")
                    nc.tensor.matmul(zb_ps[:, :], onr_sb[:, :], rz[:, :],
                                     start=True, stop=True)
                    zb_sb = rzp.tile([HD, 512], F32, tag="zbs")
                    nc.vector.tensor_copy(zb_sb[:, :], zb_ps[:, :])
                    usc = uscp.tile([HD, 512], F16)
                    with nc.allow_low_precision(reason="fp16 all-to-all payload"):
                        nc.vector.tensor_mul(usc[:, :], u_ps[0:HD, :], zb_sb[:, :])
                    nc.sync.dma_start(out=send[b][2 * n, :, :], in_=usc[:, 0:LW])
                    nc.sync.dma_start(out=send[b][2 * n + 1, :, :],
                                      in_=usc[:, LW:512])

                def a2a(b):
                    nc.gpsimd.collective_compute(
                        "AllToAll", mybir.AluOpType.bypass,
                        replica_groups=[list(range(NC))],
                        ins=[send[b].opt()], outs=[recv[b].opt()],
                    )

                def outproj(b):
                    q = nc.gpsimd if b == 0 else nc.scalar
                    q2 = nc.gpsimd if b == 0 else nc.sync
                    rvs = []
                    for dc in range(4):
                        rvh = rvp.tile([128, LW], F16, tag="rvh")
                        q.dma_start(out=rvh[0:HD, :], in_=recv[b][2 * dc, :, :])
                        q2.dma_start(out=rvh[HD:128, :],
                                     in_=recv[b][2 * dc + 1, :, :])
                        rvs.append(rvh)
                    for dt_ in range(4):
                        y_ps = pb.tile([128, 512], F32, tag="pb")
                        for dc in range(4):
                            nc.tensor.matmul(y_ps[:, 0:LW],
                                             wo_sb[:, dc, 128 * dt_:128 * (dt_ + 1)],
                                             rvs[dc][:, :], start=(dc == 0), stop=(dc == 3))
                        yt_sb = ytp.tile([128, LW], F32)
                        nc.vector.tensor_scalar_add(yt_sb[:, :], y_ps[:, 0:LW],
                                                    bo_sb[:, dt_:dt_ + 1])
                        q.dma_start(out=yts[b, 128 * dt_:128 * (dt_ + 1), :],
                                    in_=yt_sb[:, :])

                for b in range(B):
                    for n in range(NL):
                        pp_qk(b, n)
                    att_scores(b, 0)
                    for n in range(NL):
                        pp_v(b, n)
                    for n in range(1, NL):
                        att_scores(b, n)
                        att_pv(b, n - 1)
                    att_pv(b, NL - 1)
                    a2a(b)
                outproj(0)
                outproj(1)

                # ACT-queue ordering: [sqrts(b0)][exps(b0)][sqrts(b1)][exps(b1)]
                # -> the Sqrt/Exp activation-table sets load exactly 4 times.
                for b in range(B):
                    last_sq = sqrt_insts[b][-1].ins
                    for ie in exp_insts[b]:
                        add_dep_helper(ie.ins, last_sq, sync=False,
                                       reason="act-table grouping")
                if exp_insts[0]:
                    last_e0 = exp_insts[0][-1].ins
                    for isq in sqrt_insts[1]:
                        add_dep_helper(isq.ins, last_e0, sync=False,
                                       reason="act-table grouping")
    nc.compile()
    return nc


_CACHE = {}


def _get(key, repeat: int = 1):
    k = (key, repeat)
    if k not in _CACHE:
        _CACHE[k] = _build(key, repeat)
    return _CACHE[k]


def _make_w(coef):
    iu = np.triu_indices(D, k=1)
    a = np.zeros((D, D), np.float32)
    a[iu] = coef
    return a - a.T + np.eye(D, dtype=np.float32)


def _prep(x, mask, coef_q, coef_k, coef_v, coef_o,
          bias_q, bias_k, bias_v, bias_o, basis_q, basis_k):
    x = np.asarray(x, np.float32)
    mask = np.asarray(mask, np.float32)
    wq, wk, wv, wo = (_make_w(np.asarray(c, np.float32))
                      for c in (coef_q, coef_k, coef_v, coef_o))
    basis_q = np.asarray(basis_q, np.float32)
    basis_k = np.asarray(basis_k, np.float32)
    cmt = np.ascontiguousarray(
        basis_k[:HD - 1, :].T @ basis_q[:HD - 1, :]).astype(np.float32)
    xtn = np.ascontiguousarray(x.transpose(0, 2, 1))
    wot = np.ascontiguousarray(wo.T)

    bias_q = np.asarray(bias_q, np.float32)
    bias_k = np.asarray(bias_k, np.float32)
    bias_v = np.asarray(bias_v, np.float32)
    bias_o = np.asarray(bias_o, np.float32)

    # causal fast path: mask[q, k] == 0 for k <= q else -1e9
    ii = np.arange(L)
    causal_ref = np.where(ii[None, :] <= ii[:, None], 0.0, -1e9).astype(np.float32)
    causal = bool(np.array_equal(mask, causal_ref))

    on3 = np.zeros((128, 3), np.float32)
    on3[0:HD, 0] = 1.0
    on3[0:HD, 1] = 1.0
    on3[HD:128, 2] = 1.0

    shared = {
        "xt": xtn, "cmt": cmt,
        "wo4h": np.ascontiguousarray(
            wot.reshape(4, 128, D).transpose(1, 0, 2).reshape(128, 2048)
        ).astype(np.float16),
        "on3": on3,
        "onr": np.ones((1, HD), np.float32),
        "onp": np.ones((128, 1), np.float32),
        "idm": np.eye(128, dtype=np.float32),
        "bo4": np.ascontiguousarray(bias_o.reshape(4, 128).T),
    }
    if causal:
        r = np.arange(4)[:, None, None] * 128
        p = np.arange(128)[None, :, None]
        f = np.arange(512)[None, None, :]
        shared["maskd"] = np.where(f >= r + p, 0.0, -8e9).astype(np.float32)
    else:
        shared["maskf"] = np.ascontiguousarray(8.0 * mask.T)

    in_maps = []
    for c in range(NC):
        hs = slice(HD * c, HD * (c + 1))
        m = dict(shared)
        wqkt = np.concatenate([wq[hs, :].T, wk[hs, :].T], axis=1)   # [512, 128]
        m["wqk4"] = np.ascontiguousarray(
            wqkt.reshape(4, 128, 128).transpose(1, 0, 2).reshape(128, 512))
        wvt = wv[hs, :].T                                            # [512, 64]
        m["wv4"] = np.ascontiguousarray(
            wvt.reshape(4, 128, HD).transpose(1, 0, 2).reshape(128, 256))
        m["bq"] = np.ascontiguousarray(bias_q[hs, None])
        m["bk"] = np.ascontiguousarray(bias_k[hs, None])
        m["bv"] = np.ascontiguousarray(
            np.broadcast_to(bias_v[hs][None, :], (128, HD)))
        m["mukr"] = np.full((1, L), -np.linalg.norm(wk[hs, :]), np.float32)
        in_maps.append(m)
    return causal, in_maps


def kernel(_trace=False, **inputs):
    key, in_maps = _prep(**inputs)
    nc = _get(key)
    res = run_bass_kernel_spmd(nc, in_maps, list(range(NC)), trace=_trace)
    y = np.empty((B, L, D), np.float32)
    for c in range(NC):
        y[:, LW * c:LW * (c + 1), :] = res.results[c]["yts"].transpose(0, 2, 1)
    if _trace:
        kernel._last = res
    return y


def bench(inputs, repeats=(1, 5), iters=5):
    """Per-iteration HW-ish time via repeat-differencing (no NTFF here)."""
    import time as _t
    key, in_maps = _prep(**inputs)
    walls = {}
    for rep in repeats:
        nc = _get(key, rep)
        run_bass_kernel_spmd(nc, in_maps, list(range(NC)))  # warm (compile+cache)
        best = float("inf")
        for _ in range(iters):
            t0 = _t.perf_counter()
            run_bass_kernel_spmd(nc, in_maps, list(range(NC)))
            best = min(best, _t.perf_counter() - t0)
        walls[rep] = best
    r0, r1 = min(repeats), max(repeats)
    per_iter_ns = (walls[r1] - walls[r0]) / (r1 - r0) * 1e9
    return per_iter_ns, walls


# revision 27
# speedup vs baseline: 1.0233x; 1.0233x over previous
"""EquivariantAttention Trainium2 kernel (v3 — pipelined).

B=2, L=2048, D=512, H=8, HD=64 over 8 NeuronCores.
Head-parallel attention (core c owns head c, both batches), AllToAll to
sequence-shard the output projection (core c owns q-window [256c, 256c+256)).

Math notes:
  Qi . Ki = ||Q_l||*||K_m|| + Q_l^T C K_m,  C = basis_q[:63].T @ basis_k[:63]
  -> scores^T computed transposed ([k, q]) with 66-row operands:
     qt = [Qt ; ||Q|| ; ||Q||]   kt = [C Kt ; ||K|| ; -muk]
  so the contraction yields qck + ||Q||*(||K|| - muk) with no separate
  centering op.  The softmax denominator comes from an appended ones-row in V
  (row 64 of U~); softmax is max-free (scores bounded for this problem).

Structure:
  - x is DMA'd per 512-column slice ([128, 4, 512] rearranged loads) and the
    projection + attention for each slice run back-to-back, so the PE starts
    ~4us in and tracks the x DMA.
  - squares on all 128 partitions straight from the QK PSUM tile (bias-free
    fast path) + one paired ones-reduce matmul; one Sqrt writes both nq rows.
  - ACT-order dependency edges force [sqrts(b0)][exps(b0)][sqrts(b1)][exps(b1)]
    so the activation-table (Sqrt set <-> Exp set) loads exactly 4 times.
  - exp outputs fp16; V/PV and the output projection run fp16 (1 cycle/row).
  - gpsimd queue carries ONLY const DMAs + the collectives: anything else
    there would queue behind the first collective's input wait.
  - per-batch AllToAll issued as soon as that batch's attention is done,
    overlapping the other batch's compute.
"""

import sys

sys.path.insert(0, "/opt/trn_rl_repo")

import numpy as np

import concourse.bass as bass  # noqa: F401  (AP helpers)
import concourse.tile as tile
from concourse import bacc, mybir
from concourse.bass_utils import run_bass_kernel_spmd
from concourse.tile import add_dep_helper

F32 = mybir.dt.float32
F32R = mybir.dt.float32r
F16 = mybir.dt.float16
EXP = mybir.ActivationFunctionType.Exp
SQRT = mybir.ActivationFunctionType.Sqrt

B, L, D, H, HD = 2, 2048, 512, 8, 64
NC = 8
LW = L // NC          # 256: per-core q-window for the output projection
NL = 4                # l-slices of 512 per batch
NK = L // 128         # 16 k-tiles per batch
SHIFT = -20.0         # softmax global shift; exp stays f32r (score range
                      # ~[-221, 332] spans ~69 nats -- far beyond fp16)


def _build(causal: bool, repeat: int = 1):
    nc = bacc.Bacc("TRN2", target_bir_lowering=False, debug=False,
                   enable_asserts=True, num_devices=NC)

    xt = nc.dram_tensor("xt", [B, D, L], F32R, kind="ExternalInput")
    wqk4 = nc.dram_tensor("wqk4", [128, 512], F32R, kind="ExternalInput")
    wv4 = nc.dram_tensor("wv4", [128, 256], F32R, kind="ExternalInput")
    wo4h = nc.dram_tensor("wo4h", [128, 2048], F16, kind="ExternalInput")
    cmt = nc.dram_tensor("cmt", [HD, HD], F32R, kind="ExternalInput")
    on3 = nc.dram_tensor("on3", [128, 3], F32R, kind="ExternalInput")
    onr = nc.dram_tensor("onr", [1, HD], F32R, kind="ExternalInput")
    onp = nc.dram_tensor("onp", [128, 1], F32R, kind="ExternalInput")
    mukr = nc.dram_tensor("mukr", [1, L], F32R, kind="ExternalInput")
    idm = nc.dram_tensor("idm", [128, 128], F32R, kind="ExternalInput")
    if not causal:
        maskf = nc.dram_tensor("maskf", [L, L], F32, kind="ExternalInput")
    bq = nc.dram_tensor("bq", [HD, 1], F32, kind="ExternalInput")
    bk = nc.dram_tensor("bk", [HD, 1], F32, kind="ExternalInput")
    bv = nc.dram_tensor("bv", [128, HD], F32, kind="ExternalInput")
    bo4 = nc.dram_tensor("bo4", [128, 4], F32, kind="ExternalInput")
    yts = nc.dram_tensor("yts", [B, D, LW], F32, kind="ExternalOutput")

    from contextlib import ExitStack
    with tile.TileContext(nc) as tc, ExitStack() as ctx:
            ec = ctx.enter_context
            const = ec(tc.tile_pool(name="const", bufs=1))
            xtp = ec(tc.tile_pool(name="xtp", bufs=6))
            qtp = ec(tc.tile_pool(name="qtp", bufs=2))
            ktp = ec(tc.tile_pool(name="ktp", bufs=2))
            krawp = ec(tc.tile_pool(name="krawp", bufs=2))
            sqp = ec(tc.tile_pool(name="sqp", bufs=2))
            vttp = ec(tc.tile_pool(name="vttp", bufs=2))
            vp = ec(tc.tile_pool(name="vp", bufs=2))
            expp = ec(tc.tile_pool(name="expp", bufs=20))
            uscp = ec(tc.tile_pool(name="uscp", bufs=3))
            rzp = ec(tc.tile_pool(name="rzp", bufs=2))
            ssqp = ec(tc.tile_pool(name="ssqp", bufs=4))
            rvp = ec(tc.tile_pool(name="rvp", bufs=4))
            ytp = ec(tc.tile_pool(name="ytp", bufs=3))
            mskp = ec(tc.tile_pool(name="mskp", bufs=3 if not causal else 1))
            pb = ec(tc.tile_pool(name="pb", bufs=3, space="PSUM"))
            sp = ec(tc.tile_pool(name="sp", bufs=3, space="PSUM"))
            up = ec(tc.tile_pool(name="up", bufs=2, space="PSUM"))
            dram = ec(tc.tile_pool(name="dram", bufs=1, space="DRAM"))

            # ---- constants into SBUF.  scalar HWDGE queue carries the
            # weights needed first; gpsimd SWDGE the rest; sync queue is
            # reserved for x / sends / rv / yts so x starts immediately. ----
            wqk_sb = const.tile([128, 4, 128], F32R)
            wv_sb = const.tile([128, 4, HD], F32R)
            wo_sb = const.tile([128, 4, D], F16)
            cm_sb = const.tile([HD, HD], F32R)
            on3_sb = const.tile([128, 3], F32R)
            onr_sb = const.tile([1, HD], F32R)
            onp_sb = const.tile([128, 1], F32R)
            ident_sb = const.tile([128, 128], F32R)
            shift_sb = const.tile([128, 1], F32)

            nc.vector.memset(shift_sb[:, :], SHIFT)
            bq_sb = const.tile([HD, 1], F32)
            bk_sb = const.tile([HD, 1], F32)
            bv_sb = const.tile([128, HD], F32)
            bo_sb = const.tile([128, 4], F32)
            # minimal pre-x consts on sync (x starts ~2.6us); the rest on
            # scalar, whose SEQ dispatches clear before ACT's first compute
            nc.sync.dma_start(out=wqk_sb[:, :, :],
                              in_=wqk4[:, :].rearrange("p (c m) -> p c m", c=4))
            nc.sync.dma_start(out=bq_sb[:, :], in_=bq[:, :])
            nc.sync.dma_start(out=bk_sb[:, :], in_=bk[:, :])
            nc.scalar.dma_start(out=on3_sb[:, :], in_=on3[:, :])
            nc.scalar.dma_start(out=cm_sb[:, :], in_=cmt[:, :])
            # bulk / later-needed consts on the scalar HWDGE queue
            nc.scalar.dma_start(out=wv_sb[:, :, :],
                                in_=wv4[:, :].rearrange("p (c m) -> p c m", c=4))
            nc.scalar.dma_start(out=ident_sb[:, :], in_=idm[:, :])
            nc.scalar.dma_start(out=bv_sb[:, :], in_=bv[:, :])
            nc.scalar.dma_start(out=onr_sb[:, :], in_=onr[:, :])
            nc.scalar.dma_start(out=onp_sb[:, :], in_=onp[:, :])
            nc.scalar.dma_start(out=wo_sb[:, :, :],
                                in_=wo4h[:, :].rearrange("p (c m) -> p c m", c=4))
            nc.scalar.dma_start(out=bo_sb[:, :], in_=bo4[:, :])

            for _rep in range(repeat):
                send = [dram.tile([NC, HD, LW], F16, tag=f"send{b}", name=f"send{b}") for b in range(B)]
                recv = [dram.tile([NC, HD, LW], F16, tag=f"recv{b}", name=f"recv{b}") for b in range(B)]

                # per-batch tiles + their mukr rows first (tiny, gates the
                # score matmuls), then all x loads on the sync queue
                state = {}
                for b in range(B):
                    qt = qtp.tile([HD + 2, L], F32R)   # [Qt ; ||Q|| ; ||Q||]
                    kt = ktp.tile([HD + 2, L], F32R)   # [C Kt ; ||K|| ; -muk]
                    kraw = krawp.tile([HD, L], F32R)
                    vtt = vttp.tile([HD, L], F32R)
                    vt = vp.tile([128, NK, HD + 1], F32R)
                    nc.sync.dma_start(out=kt[HD + 1:HD + 2, :], in_=mukr[:, :])
                    state[b] = (qt, kt, kraw, vtt, vt)
                xts = {}
                for b in range(B):
                    for ls in range(NL):
                        t = xtp.tile([128, 4, 512], F32R, tag="xt")
                        nc.sync.dma_start(
                            out=t[:, :, :],
                            in_=xt[b, :, 512 * ls:512 * (ls + 1)]
                                .rearrange("(c p) m -> p c m", p=128))
                        xts[(b, ls)] = t

                sqrt_insts = {0: [], 1: []}
                exp_insts = {0: [], 1: []}

                def pp_qk(b, ls):
                    qt, kt, kraw, vtt, vt = state[b]
                    s = slice(512 * ls, 512 * (ls + 1))
                    xls = xts[(b, ls)]
                    qk_ps = pb.tile([128, 512], F32, tag="pb")
                    for dc in range(4):
                        nc.tensor.matmul(qk_ps[:, :], wqk_sb[:, dc, :],
                                         xls[:, dc, :],
                                         start=(dc == 0), stop=(dc == 3))
                    nc.vector.tensor_scalar_add(qt[0:HD, s], qk_ps[0:HD, :],
                                                bq_sb[:, 0:1])
                    nc.vector.tensor_scalar_add(kraw[:, s], qk_ps[HD:128, :],
                                                bk_sb[:, 0:1])
                    sq2 = sqp.tile([128, 512], F32R, tag="sq")
                    nc.vector.tensor_mul(sq2[0:HD, :], qt[0:HD, s], qt[0:HD, s])
                    nc.vector.tensor_mul(sq2[HD:128, :], kraw[:, s], kraw[:, s])
                    # paired column-sums; one Sqrt fills both nq rows of qt,
                    # one fills kt's nk row (PSUM reads must start at part 0).
                    ssqq_ps = pb.tile([2, 512], F32, tag="pb")
                    nc.tensor.matmul(ssqq_ps[:, :], on3_sb[:, 0:2], sq2[:, :],
                                     start=True, stop=True)
                    ssqk_ps = pb.tile([1, 512], F32, tag="pb")
                    nc.tensor.matmul(ssqk_ps[:, :], on3_sb[:, 2:3], sq2[:, :],
                                     start=True, stop=True)
                    if b == 0:
                        i1 = nc.scalar.activation(qt[HD:HD + 2, s], ssqq_ps[:, :],
                                                  SQRT)
                        i2 = nc.scalar.activation(kt[HD:HD + 1, s], ssqk_ps[:, :],
                                                  SQRT)
                    else:
                        # b1's projection runs during b0's attention, but its
                        # sqrts are ACT-ordered after b0's exps: bounce ssq to
                        # SBUF (idle DVE) so the PSUM slots free immediately
                        ssqq_sb = ssqp.tile([2, 512], F32, tag="ssqq")
                        ssqk_sb = ssqp.tile([1, 512], F32, tag="ssqk")
                        nc.vector.tensor_copy(ssqq_sb[:, :], ssqq_ps[:, :])
                        nc.vector.tensor_copy(ssqk_sb[:, :], ssqk_ps[:, :])
                        i1 = nc.scalar.activation(qt[HD:HD + 2, s], ssqq_sb[:, :],
                                                  SQRT)
                        i2 = nc.scalar.activation(kt[HD:HD + 1, s], ssqk_sb[:, :],
                                                  SQRT)
                    sqrt_insts[b] += [i1, i2]
                    # CK projection (kt needed by the first score tiles)
                    ck_ps = pb.tile([128, 512], F32, tag="pb")
                    nc.tensor.matmul(ck_ps[0:HD, :], cm_sb[:, :], kraw[:, s],
                                     start=True, stop=True)
                    # b0: ACT is idle during its projection phase; b1's kt
                    # copies run while ACT streams b0's exps -> DVE instead
                    if b == 0:
                        nc.scalar.copy(kt[0:HD, s], ck_ps[0:HD, :])
                    else:
                        nc.vector.tensor_copy(kt[0:HD, s], ck_ps[0:HD, :])

                def pp_v(b, ls):
                    qt, kt, kraw, vtt, vt = state[b]
                    s = slice(512 * ls, 512 * (ls + 1))
                    xls = xts[(b, ls)]
                    # V (transposed projection), then PE transpose to rows
                    vt_ps = pb.tile([128, 512], F32, tag="pb")
                    for dc in range(4):
                        nc.tensor.matmul(vt_ps[0:HD, :], wv_sb[:, dc, :],
                                         xls[:, dc, :],
                                         start=(dc == 0), stop=(dc == 3))
                    nc.vector.tensor_copy(vtt[:, s], vt_ps[0:HD, :])
                    tp_ps = pb.tile([128, 4, HD], F32, tag="pb")
                    for j in range(4):
                        lt = 4 * ls + j
                        nc.tensor.transpose(tp_ps[:, j, :].bitcast(F32R),
                                            vtt[:, 128 * lt:128 * (lt + 1)],
                                            ident_sb[0:HD, 0:HD])
                    for j in range(4):
                        nc.vector.tensor_add(vt[:, 4 * ls + j, 0:HD],
                                             tp_ps[:, j, :], bv_sb[:, :])
                        nc.vector.tensor_copy(vt[:, 4 * ls + j, HD:HD + 1],
                                              onp_sb[:, :])

                ex_tiles = {}

                def _kis(n):
                    kmax = 4 * (n + 1) if causal else NK
                    return list(range(kmax))

                def _win(n, ki):
                    # diagonal k-tiles only need q-cols >= 128r within the
                    # window; ki==0/first is always full width (start=True)
                    # so trimmed tiles overwrite cleanly.
                    lo = 0
                    if causal and 4 * n <= ki <= 4 * n + 3:
                        lo = 128 * (ki - 4 * n)
                    return slice(lo, 512)

                def att_scores(b, n):
                    qt, kt, kraw, vtt, vt = state[b]
                    qs = slice(512 * n, 512 * (n + 1))
                    for ki in _kis(n):
                        w = _win(n, ki)
                        st_ps = sp.tile([128, 512], F32, tag="sp")
                        nc.tensor.matmul(st_ps[:, w],
                                         kt[:, 128 * ki:128 * (ki + 1)],
                                         qt[:, qs][:, w], start=True, stop=True)
                        if not causal:
                            mld = mskp.tile([128, 512], F32)
                            nc.sync.dma_start(out=mld[:, :],
                                              in_=maskf[128 * ki:128 * (ki + 1), qs])
                            nc.vector.tensor_add(st_ps[:, :], st_ps[:, :], mld[:, :])
                        ex = expp.tile([128, 512], F32R)
                        ie = nc.scalar.activation(ex[:, w], st_ps[:, w], EXP,
                                                  scale=0.125,
                                                  bias=shift_sb[:, 0:1])
                        exp_insts[b].append(ie)
                        if causal and 4 * n <= ki <= 4 * n + 3:
                            # zero the strictly-upper triangle of the diagonal
                            # block on the idle gpsimd engine (post-exp)
                            r = ki - 4 * n
                            ds_ = slice(128 * r, 128 * (r + 1))
                            nc.gpsimd.affine_select(
                                ex[:, ds_], ex[:, ds_],
                                pattern=[[1, 128]],
                                compare_op=mybir.AluOpType.is_ge,
                                fill=0.0, base=0, channel_multiplier=-1)
                        ex_tiles[(b, n, ki)] = ex

                def att_pv(b, n):
                    qt, kt, kraw, vtt, vt = state[b]
                    kis = _kis(n)
                    first_ki, last_ki = kis[0], kis[-1]
                    u_ps = up.tile([HD + 1, 512], F32, tag="up")
                    for ki in kis:
                        w = _win(n, ki)
                        ex = ex_tiles.pop((b, n, ki))
                        nc.tensor.matmul(u_ps[:, w], vt[:, ki, :], ex[:, w],
                                         start=(ki == first_ki),
                                         stop=(ki == last_ki))
                    rz = rzp.tile([1, 512], F32R, tag="rz")
                    with nc.allow_low_precision(reason="f32r rounding of softmax denom"):
                        nc.vector.reciprocal(rz[:, :], u_ps[HD:HD + 1, :])
                    zb_ps = pb.tile([HD, 512], F32, tag="pb")
                    nc.tensor.matmul(zb_ps[:, :], onr_sb[:, :], rz[:, :],
                                     start=True, stop=True)
                    zb_sb = rzp.tile([HD, 512], F32, tag="zbs")
                    nc.vector.tensor_copy(zb_sb[:, :], zb_ps[:, :])
                    usc = uscp.tile([HD, 512], F16)
                    with nc.allow_low_precision(reason="fp16 all-to-all payload"):
                        nc.vector.tensor_mul(usc[:, :], u_ps[0:HD, :], zb_sb[:, :])
                    nc.sync.dma_start(out=send[b][2 * n, :, :], in_=usc[:, 0:LW])
                    nc.sync.dma_start(out=send[b][2 * n + 1, :, :],
                                      in_=usc[:, LW:512])

                def a2a(b):
                    nc.gpsimd.collective_compute(
                        "AllToAll", mybir.AluOpType.bypass,
                        replica_groups=[list(range(NC))],
                        ins=[send[b].opt()], outs=[recv[b].opt()],
                    )

                def outproj(b):
                    q = nc.gpsimd if b == 0 else nc.scalar
                    q2 = nc.gpsimd if b == 0 else nc.sync
                    rvs = []
                    for dc in range(4):
                        rvh = rvp.tile([128, LW], F16, tag="rvh")
                        q.dma_start(out=rvh[0:HD, :], in_=recv[b][2 * dc, :, :])
                        q2.dma_start(out=rvh[HD:128, :],
                                     in_=recv[b][2 * dc + 1, :, :])
                        rvs.append(rvh)
                    for dt_ in range(4):
                        y_ps = pb.tile([128, 512], F32, tag="pb")
                        for dc in range(4):
                            nc.tensor.matmul(y_ps[:, 0:LW],
                                             wo_sb[:, dc, 128 * dt_:128 * (dt_ + 1)],
                                             rvs[dc][:, :], start=(dc == 0), stop=(dc == 3))
                        yt_sb = ytp.tile([128, LW], F32)
                        nc.vector.tensor_scalar_add(yt_sb[:, :], y_ps[:, 0:LW],
                                                    bo_sb[:, dt_:dt_ + 1])
                        q.dma_start(out=yts[b, 128 * dt_:128 * (dt_ + 1), :],
                                    in_=yt_sb[:, :])

                for n in range(NL):
                    pp_qk(0, n)
                att_scores(0, 0)
                for n in range(NL):
                    pp_v(0, n)
                att_scores(0, 1)
                att_pv(0, 0)
                pp_qk(1, 0)
                att_scores(0, 2)
                att_pv(0, 1)
                pp_qk(1, 1)
                att_scores(0, 3)
                att_pv(0, 2)
                pp_qk(1, 2)
                att_pv(0, 3)
                pp_qk(1, 3)
                a2a(0)
                att_scores(1, 0)
                for n in range(NL):
                    pp_v(1, n)
                for n in range(1, NL):
                    att_scores(1, n)
                    att_pv(1, n - 1)
                att_pv(1, NL - 1)
                a2a(1)
                outproj(0)
                outproj(1)

                # ACT-queue ordering: [sqrts(b0)][exps(b0)][sqrts(b1)][exps(b1)]
                # -> the Sqrt/Exp activation-table sets load exactly 4 times.
                for b in range(B):
                    last_sq = sqrt_insts[b][-1].ins
                    for ie in exp_insts[b]:
                        add_dep_helper(ie.ins, last_sq, sync=False,
                                       reason="act-table grouping")
                if exp_insts[0]:
                    last_e0 = exp_insts[0][-1].ins
                    for isq in sqrt_insts[1]:
                        add_dep_helper(isq.ins, last_e0, sync=False,
                                       reason="act-table grouping")
    nc.compile()
    return nc


_CACHE = {}


def _get(key, repeat: int = 1):
    k = (key, repeat)
    if k not in _CACHE:
        _CACHE[k] = _build(key, repeat)
    return _CACHE[k]


def _make_w(coef):
    iu = np.triu_indices(D, k=1)
    a = np.zeros((D, D), np.float32)
    a[iu] = coef
    return a - a.T + np.eye(D, dtype=np.float32)


def _prep(x, mask, coef_q, coef_k, coef_v, coef_o,
          bias_q, bias_k, bias_v, bias_o, basis_q, basis_k):
    x = np.asarray(x, np.float32)
    mask = np.asarray(mask, np.float32)
    wq, wk, wv, wo = (_make_w(np.asarray(c, np.float32))
                      for c in (coef_q, coef_k, coef_v, coef_o))
    basis_q = np.asarray(basis_q, np.float32)
    basis_k = np.asarray(basis_k, np.float32)
    cmt = np.ascontiguousarray(
        basis_k[:HD - 1, :].T @ basis_q[:HD - 1, :]).astype(np.float32)
    xtn = np.ascontiguousarray(x.transpose(0, 2, 1))
    wot = np.ascontiguousarray(wo.T)

    bias_q = np.asarray(bias_q, np.float32)
    bias_k = np.asarray(bias_k, np.float32)
    bias_v = np.asarray(bias_v, np.float32)
    bias_o = np.asarray(bias_o, np.float32)

    # causal fast path: mask[q, k] == 0 for k <= q else -1e9
    ii = np.arange(L)
    causal_ref = np.where(ii[None, :] <= ii[:, None], 0.0, -1e9).astype(np.float32)
    causal = bool(np.array_equal(mask, causal_ref))

    on3 = np.zeros((128, 3), np.float32)
    on3[0:HD, 0] = 1.0
    on3[0:HD, 1] = 1.0
    on3[HD:128, 2] = 1.0

    shared = {
        "xt": xtn, "cmt": cmt,
        "wo4h": np.ascontiguousarray(
            wot.reshape(4, 128, D).transpose(1, 0, 2).reshape(128, 2048)
        ).astype(np.float16),
        "on3": on3,
        "onr": np.ones((1, HD), np.float32),
        "onp": np.ones((128, 1), np.float32),
        "idm": np.eye(128, dtype=np.float32),
        "bo4": np.ascontiguousarray(bias_o.reshape(4, 128).T),
    }
    if causal:
        r = np.arange(4)[:, None, None] * 128
        p = np.arange(128)[None, :, None]
        f = np.arange(512)[None, None, :]
        shared["maskd"] = np.where(f >= r + p, 0.0, -8e9).astype(np.float32)
    else:
        shared["maskf"] = np.ascontiguousarray(8.0 * mask.T)

    in_maps = []
    for c in range(NC):
        hs = slice(HD * c, HD * (c + 1))
        m = dict(shared)
        wqkt = np.concatenate([wq[hs, :].T, wk[hs, :].T], axis=1)   # [512, 128]
        m["wqk4"] = np.ascontiguousarray(
            wqkt.reshape(4, 128, 128).transpose(1, 0, 2).reshape(128, 512))
        wvt = wv[hs, :].T                                            # [512, 64]
        m["wv4"] = np.ascontiguousarray(
            wvt.reshape(4, 128, HD).transpose(1, 0, 2).reshape(128, 256))
        m["bq"] = np.ascontiguousarray(bias_q[hs, None])
        m["bk"] = np.ascontiguousarray(bias_k[hs, None])
        m["bv"] = np.ascontiguousarray(
            np.broadcast_to(bias_v[hs][None, :], (128, HD)))
        m["mukr"] = np.full((1, L), -np.linalg.norm(wk[hs, :]), np.float32)
        in_maps.append(m)
    return causal, in_maps


def kernel(_trace=False, **inputs):
    key, in_maps = _prep(**inputs)
    nc = _get(key)
    res = run_bass_kernel_spmd(nc, in_maps, list(range(NC)), trace=_trace)
    y = np.empty((B, L, D), np.float32)
    for c in range(NC):
        y[:, LW * c:LW * (c + 1), :] = res.results[c]["yts"].transpose(0, 2, 1)
    if _trace:
        kernel._last = res
    return y


def bench(inputs, repeats=(1, 5), iters=5):
    """Per-iteration HW-ish time via repeat-differencing (no NTFF here)."""
    import time as _t
    key, in_maps = _prep(**inputs)
    walls = {}
    for rep in repeats:
        nc = _get(key, rep)
        run_bass_kernel_spmd(nc, in_maps, list(range(NC)))  # warm (compile+cache)
        best = float("inf")
        for _ in range(iters):
            t0 = _t.perf_counter()
            run_bass_kernel_spmd(nc, in_maps, list(range(NC)))
            best = min(best, _t.perf_counter() - t0)
        walls[rep] = best
    r0, r1 = min(repeats), max(repeats)
    per_iter_ns = (walls[r1] - walls[r0]) / (r1 - r0) * 1e9
    return per_iter_ns, walls
